# revision 28
# baseline (speedup 1.0000x reference)
"""Trainium2 Bass kernel for conv-stack + NetVLAD pooling + linear head (v2).

Pure data parallel: 32 images sharded 4-per-core across 8 NeuronCores.

v2 structural changes vs v1:
  - x loaded with 4 giant dma_starts per image (descriptors spread across
    all 16 DMA engines at ~220 GB/s vs ~4 engine-streams before) into a
    persistent whole-image SBUF tile x1all[(a,ci), 17 tiles, 514].
  - conv2 staging: 2 big strided SBUF->SBUF DMAs per image instead of 32.
  - NetVLAD: per 128-w tile ONE f32r matmul with moving [P | Ablk] computes
    the pair-summed transposed features xft AND the logits; no PE
    transposes, no bf16 V copy. Softmax batched once per image.
  - Finale batched across the 4 images, once per core.
  - relu+pool work split across ACT / DVE / Pool(gpsimd) engines.
"""
import sys

sys.path.insert(0, "/opt/trn_rl_repo")

import numpy as np
import concourse.bacc as bacc
import concourse.tile as tile
from concourse import mybir
from concourse.bass_utils import run_bass_kernel_spmd

F32 = mybir.dt.float32
F32R = mybir.dt.float32r
BF16 = mybir.dt.bfloat16
AX = mybir.AxisListType
ALU = mybir.AluOpType
ACTF = mybir.ActivationFunctionType

N_CORES = 8
IPC = 4  # images per core
EPS = 1e-12


def _round_f32r(a):
    u = np.ascontiguousarray(a, np.float32).view(np.uint32)
    u = (u + 0x200) & np.uint32(0xFFFFFC00)
    return u.view(np.float32)


def _build_consts(conv1_w, conv2_w, assign_w, assign_b, lin_w, lin_b):
    c1w = np.asarray(conv1_w, np.float32)
    c2w = np.asarray(conv2_w, np.float32)
    # conv1 banded lhsT, 3 variants (var*3+dx slots):
    #  var0 (mid, tiles r=1..15): x anchor h=32r-3+a, outputs j=h_out-(32r-2)
    #    rows p=(j+dy)*3+ci, cols q=j*4+co
    #  var1 (r=0): x anchor h=a, outputs cols q=j*4+co with h_out=j-2
    #    (cols j<2 zero), taps at a=j-3+dy >= 0
    #  var2 (r=16): x anchor h=478+a, outputs h_out=510+jj at cols jj*4+co,
    #    taps at a=31+jj+dy <= 33
    W1 = np.zeros((102, 9, 128), np.float32)
    for dx in range(3):
        for co in range(4):
            for ci in range(3):
                for dy in range(3):
                    for j in range(32):
                        W1[(j + dy) * 3 + ci, dx, j * 4 + co] = c1w[co, ci, dy, dx]
                    for j in range(2, 32):
                        a = j - 3 + dy
                        if a >= 0:
                            W1[a * 3 + ci, 3 + dx, j * 4 + co] = c1w[co, ci, dy, dx]
                    for jj in range(2):
                        a = 31 + jj + dy
                        if a <= 33:
                            W1[a * 3 + ci, 6 + dx, jj * 4 + co] = c1w[co, ci, dy, dx]
    # conv2 banded lhsT with pool1-h fold (input rows are unpooled Y1 rows)
    # and the w-pools stored as SUMs: total scale 0.25.
    W2 = np.zeros((80, 3, 128), np.float32)
    for dx in range(3):
        for co in range(16):
            for ci in range(4):
                for dy in range(3):
                    for rr in range(8):
                        for half in range(2):
                            W2[(2 * rr + 2 * dy + half) * 4 + ci, dx, rr * 16 + co] = (
                                0.25 * c2w[co, ci, dy, dx]
                            )
    # combined moving matrix M = [P | Ablk]:
    #  cols 0:64  : P[(2q+h)*16+c, q*16+c] = 1   (xft pair-sum)
    #  cols 64:80 : Ablk[(2q+h)*16+c, q*4+k] = 0.25*aw[k,c]  (pooled logits)
    aw = np.asarray(assign_w, np.float32)
    M = np.zeros((128, 80), np.float32)
    for q in range(4):
        for c in range(16):
            for h in range(2):
                M[(2 * q + h) * 16 + c, q * 16 + c] = 1.0
                for k in range(4):
                    M[(2 * q + h) * 16 + c, 64 + q * 4 + k] = 0.25 * aw[k, c]
    brep = np.tile(np.asarray(assign_b, np.float32), 4).reshape(16)
    brep128 = np.broadcast_to(brep, (128, 16)).copy()
    linb4 = np.broadcast_to(np.asarray(lin_b, np.float32), (4, 7)).copy()
    return {
        "w1": _round_f32r(W1),
        "w2": _round_f32r(W2),
        "m": _round_f32r(M),
        "brep": brep128.astype(np.float32),
        "cent": np.zeros(0),  # set by caller (4x centroids)
        "wlin": np.asarray(lin_w, np.float32).T.copy(),  # (64, 7)
        "linb4": linb4,
        "ones41": np.ones((4, 1), np.float32),
        "ones44": np.ones((4, 4), np.float32),
    }


def _build_program(debug=False):
    nc = bacc.Bacc("TRN2", target_bir_lowering=False, debug=False,
                   num_devices=N_CORES)
    xin = nc.dram_tensor("x", [IPC, 3, 512, 512], F32R, kind="ExternalInput").ap()
    w1 = nc.dram_tensor("w1", [102, 9, 128], F32R, kind="ExternalInput").ap()
    w2 = nc.dram_tensor("w2", [80, 3, 128], F32R, kind="ExternalInput").ap()
    m_in = nc.dram_tensor("m", [128, 80], F32R, kind="ExternalInput").ap()
    brep = nc.dram_tensor("brep", [128, 16], F32, kind="ExternalInput").ap()
    cent = nc.dram_tensor("cent", [4, 16], F32, kind="ExternalInput").ap()
    wlin = nc.dram_tensor("wlin", [64, 7], F32, kind="ExternalInput").ap()
    linb4 = nc.dram_tensor("linb4", [4, 7], F32, kind="ExternalInput").ap()
    ones41 = nc.dram_tensor("ones41", [4, 1], F32, kind="ExternalInput").ap()
    ones44 = nc.dram_tensor("ones44", [4, 4], F32, kind="ExternalInput").ap()
    out = nc.dram_tensor("out", [IPC, 7], F32, kind="ExternalOutput").ap()
    if debug:
        y1d = nc.dram_tensor("y1d", [IPC, 128, 17, 258], F32R, kind="ExternalOutput").ap()
        vd = nc.dram_tensor("vd", [IPC, 128, 32, 128], F32R, kind="ExternalOutput").ap()
        xad = nc.dram_tensor("xad", [IPC, 102, 17, 514], F32R, kind="ExternalOutput").ap()
        lbd = nc.dram_tensor("lbd", [IPC, 128, 32, 16], F32, kind="ExternalOutput").ap()
        galld = nc.dram_tensor("galld", [4, 4, 17], F32, kind="ExternalOutput").ap()

    from contextlib import ExitStack

    with tile.TileContext(nc) as tc, ExitStack() as es:
        consts = es.enter_context(tc.tile_pool(name="consts", bufs=1))
        vp = es.enter_context(tc.tile_pool(name="vp", bufs=2))
        re1p = es.enter_context(tc.tile_pool(name="re1p", bufs=3))
        re2p = es.enter_context(tc.tile_pool(name="re2p", bufs=4))
        smp = es.enter_context(tc.tile_pool(name="smp", bufs=2))
        finp = es.enter_context(tc.tile_pool(name="finp", bufs=1))
        cvp = es.enter_context(tc.tile_pool(name="cvp", bufs=2, space="PSUM"))
        xltp = es.enter_context(tc.tile_pool(name="xltp", bufs=2, space="PSUM"))
        gp = es.enter_context(tc.tile_pool(name="gp", bufs=1, space="PSUM"))

        w1_sb = consts.tile([102, 9, 128], F32R)
        nc.scalar.dma_start(out=w1_sb, in_=w1)
        w2_sb = consts.tile([80, 3, 128], F32R)
        nc.scalar.dma_start(out=w2_sb, in_=w2)
        w2bf_sb = consts.tile([80, 3, 128], BF16)
        nc.vector.tensor_copy(w2bf_sb, w2_sb.bitcast(F32))
        m_sb = consts.tile([128, 80], F32R)
        nc.scalar.dma_start(out=m_sb, in_=m_in)
        mbf_sb = consts.tile([128, 80], BF16)
        nc.vector.tensor_copy(mbf_sb, m_sb.bitcast(F32))
        brep_sb = consts.tile([128, 16], F32)
        nc.scalar.dma_start(out=brep_sb, in_=brep)
        cent_sb = consts.tile([4, 16], F32)
        nc.scalar.dma_start(out=cent_sb, in_=cent)
        wlin_sb = consts.tile([64, 7], F32)
        nc.scalar.dma_start(out=wlin_sb, in_=wlin)
        linb4_sb = consts.tile([4, 7], F32)
        nc.scalar.dma_start(out=linb4_sb, in_=linb4)
        ones41_sb = consts.tile([4, 1], F32)
        nc.scalar.dma_start(out=ones41_sb, in_=ones41)
        ones44_sb = consts.tile([4, 4], F32)
        nc.scalar.dma_start(out=ones44_sb, in_=ones44)

        # persistent tiles with pre-initialized constant regions
        xa = consts.tile([102, 17, 514], F32R, tag="xa")
        nc.vector.memset(xa[:, :, 0:1].bitcast(F32), 0.0)
        nc.vector.memset(xa[:, :, 513:514].bitcast(F32), 0.0)
        xav = xa.rearrange("(a c) r w -> a c r w", c=3)
        xfts = []
        apads = []
        for half in range(2):
            xft = consts.tile([128, 32, 4, 17], BF16, tag=f"xft{half}")
            nc.vector.memset(xft[:, :, :, 16:17], 1.0)
            xfts.append(xft)
            apad = consts.tile([128, 32, 4, 32], BF16, tag=f"apad{half}")
            nc.vector.memset(apad[:, :, :, 4:32], 0.0)
            apads.append(apad)

        x2 = consts.tile([80, 16, 258], BF16, tag="x2")
        y1s = []
        for half in range(2):
            y1t = consts.tile([128, 17, 258], BF16, tag=f"y1{half}")
            nc.vector.memset(y1t[:, :, 0:1], 0.0)
            nc.vector.memset(y1t[:, :, 257:258], 0.0)
            y1s.append(y1t)

        gall = finp.tile([4, 4, 17], F32, tag="gall")
        pending_gram = []

        for img in range(IPC):
            # ================== x load: 4 giant DMAs ==================
            # r=0 tile: anchor h=a (rows 0..33)
            for ci in range(3):
                nc.sync.dma_start(
                    out=xav[:, ci, 0, 1:513], in_=xin[img, ci, 0:34, :]
                )
            # mid tiles r=1..15, a=2..33: rows 31..510 partitioned exactly
            # (DMA APs are limited to 3 dims, so one dma_start per channel)
            chunks = [(1, 4), (4, 16)] if img == 0 else [(1, 16)]
            for ci in range(3):
                for lo, hi in chunks:
                    nc.sync.dma_start(
                        out=xav[2:34, ci, lo:hi, 1:513],
                        in_=xin[img, ci, 32 * lo - 1 : 32 * (hi - 1) + 31, :]
                        .rearrange("(r a) w -> a r w", a=32),
                    )
                # mid tiles r=1..15, a=0..1: rows 32r-3, 32r-2
                nc.sync.dma_start(
                    out=xav[0:2, ci, 1:16, 1:513],
                    in_=xin[img, ci, 29:509, :].rearrange("(r k) w -> k r w", k=32)[0:2],
                )
            # r=16 tile: anchor h=478+a
            for ci in range(3):
                nc.sync.dma_start(
                    out=xav[:, ci, 16, 1:513], in_=xin[img, ci, 478:512, :]
                )

            # ================== conv1: 8 pairs + tail ==================
            y1 = y1s[img % 2]
            for pi in range(8):
                p1 = cvp.tile([128, 2, 512], F32, tag="cv")
                for j in range(2):
                    r = 2 * pi + j
                    var = 1 if r == 0 else 0
                    for dx in range(3):
                        nc.tensor.matmul(
                            p1[:, j, :], w1_sb[:, var * 3 + dx, :],
                            xa[:, r, dx : dx + 512],
                            start=(dx == 0), stop=(dx == 2),
                        )
                re1 = re1p.tile([128, 2, 512], F32, tag="re1")
                nc.scalar.activation(out=re1, in_=p1, func=ACTF.Relu)
                re1v = re1.rearrange("p a (w two) -> p a w two", two=2)
                nc.vector.tensor_add(
                    y1[:, 2 * pi : 2 * pi + 2, 1:257],
                    re1v[:, :, :, 0], re1v[:, :, :, 1],
                )
                if pi == 4:
                    # first staging half: x2 slots 0..7 need y1 blocks 0..8
                    nc.sync.dma_start(out=x2[0:64, 0:8, :], in_=y1[64:128, 0:8, :])
                    nc.sync.dma_start(out=x2[64:80, 0:8, :], in_=y1[0:16, 1:9, :])
            # tail tile r=16 (writes full 128 partitions: zeros outside rows
            # 510/511, giving conv2 its zero padding rows)
            p1t = cvp.tile([128, 2, 512], F32, tag="cv")
            for dx in range(3):
                nc.tensor.matmul(
                    p1t[:, 0, :], w1_sb[:, 6 + dx, :], xa[:, 16, dx : dx + 512],
                    start=(dx == 0), stop=(dx == 2),
                )
            re1t = re1p.tile([128, 2, 512], F32, tag="re1")
            nc.scalar.activation(out=re1t[:, 0, :], in_=p1t[:, 0, :], func=ACTF.Relu)
            re1tv = re1t.rearrange("p a (w two) -> p a w two", two=2)
            nc.vector.tensor_add(
                y1[:, 16:17, 1:257], re1tv[:, 0:1, :, 0], re1tv[:, 0:1, :, 1]
            )
            # second staging half: x2 slots 8..15 need y1 blocks 8..16
            nc.sync.dma_start(out=x2[0:64, 8:16, :], in_=y1[64:128, 8:16, :])
            nc.sync.dma_start(out=x2[64:80, 8:16, :], in_=y1[0:16, 9:17, :])

            # ========= conv2 (8 groups) interleaved with NetVLAD xlt =========
            v = vp.tile([128, 32, 128], BF16, tag="v")
            v2v = v.rearrange("p (t two) w -> p t two w", two=2)
            xft = xfts[img % 2]
            apad = apads[img % 2]
            lb = smp.tile([128, 32, 16], F32, tag="lb")

            def conv2_group(g, parity):
                p2 = cvp.tile([128, 2, 512], F32, tag="cv")
                p2v = p2.rearrange("p a (b w two) -> p (a b) w two", two=2, b=2)
                rhs = (y1[0:80] if parity == 0 else x2)
                for pr in range(2):
                    blk = 4 * g + 2 * pr
                    for dx in range(3):
                        nc.tensor.matmul(
                            p2.rearrange("p a (b w) -> p (a b) w", b=2)[
                                :, 2 * pr : 2 * pr + 2, :],
                            w2bf_sb[:, dx, :],
                            rhs[:, blk : blk + 2, dx : dx + 256],
                            start=(dx == 0), stop=(dx == 2),
                        )
                re2 = re2p.tile([128, 4, 128], F32, tag="re2")
                nc.scalar.activation(out=re2, in_=p2v[:, :, :, 0], func=ACTF.Relu)
                nc.vector.scalar_tensor_tensor(
                    out=v2v[:, 4 * g : 4 * g + 4, parity, :],
                    in0=p2v[:, :, :, 1], scalar=0.0, in1=re2,
                    op0=ALU.max, op1=ALU.add,
                )

            def xlt_group(g8):
                xlt = xltp.tile([128, 4, 80], F32, tag="xlt")
                for tt in range(4):
                    nc.tensor.matmul(
                        xlt[:, tt, :], v[:, 4 * g8 + tt, :], mbf_sb[:],
                        start=True, stop=True,
                    )
                nc.vector.tensor_copy(
                    xft[:, 4 * g8 : 4 * g8 + 4, :, 0:16],
                    xlt[:, :, 0:64].rearrange("p t (q c) -> p t q c", q=4),
                )
                nc.vector.tensor_add(
                    lb[:, 4 * g8 : 4 * g8 + 4, :],
                    xlt[:, :, 64:80],
                    brep_sb[:].unsqueeze(1).broadcast_to((128, 4, 16)),
                )

            def softmax_group(g8):
                # tiles 4*g8..4*g8+3, right after their lb is produced
                lbv = lb[:, 4 * g8 : 4 * g8 + 4, :].rearrange(
                    "p t (q k) -> p t q k", k=4)
                mx = smp.tile([128, 4, 4], F32, tag="mx")
                nc.vector.reduce_max(mx, lbv, axis=AX.X)
                ls = smp.tile([128, 4, 4, 4], F32, tag="ls")
                nc.vector.tensor_sub(
                    ls, lbv, mx.unsqueeze(-1).broadcast_to((128, 4, 4, 4)))
                ae = smp.tile([128, 4, 4, 4], F32, tag="ae")
                nc.scalar.activation(out=ae, in_=ls, func=ACTF.Exp)
                zs = smp.tile([128, 4, 4], F32, tag="zs")
                nc.vector.reduce_sum(zs, ae, axis=AX.X)
                rz = smp.tile([128, 4, 4], F32, tag="rz")
                nc.vector.reciprocal(rz, zs)
                nc.vector.scalar_tensor_tensor(
                    out=apad[:, 4 * g8 : 4 * g8 + 4, :, 0:4], in0=ae,
                    scalar=0.25,
                    in1=rz.unsqueeze(-1).broadcast_to((128, 4, 4, 4)),
                    op0=ALU.mult, op1=ALU.mult,
                )

            conv2_group(0, 0)
            conv2_group(0, 1)
            # flush previous image's deferred gram here: apad of image i-1 is
            # ready by now and conv1+2 groups keep the PE fed meanwhile
            for fn in pending_gram:
                fn()
            pending_gram.clear()
            xlt_group(0)
            softmax_group(0)
            xlt_group(1)
            softmax_group(1)
            conv2_group(1, 0)
            conv2_group(1, 1)
            xlt_group(2)
            softmax_group(2)
            xlt_group(3)
            softmax_group(3)
            conv2_group(2, 0)
            conv2_group(2, 1)
            xlt_group(4)
            softmax_group(4)
            xlt_group(5)
            softmax_group(5)
            conv2_group(3, 0)
            conv2_group(3, 1)
            xlt_group(6)
            softmax_group(6)
            xlt_group(7)
            softmax_group(7)
            if debug:
                nc.scalar.dma_start(out=xad[img], in_=xa)
                nc.scalar.dma_start(out=y1d[img], in_=y1)
                nc.scalar.dma_start(out=vd[img], in_=v)
                nc.scalar.dma_start(out=lbd[img], in_=lb)

            # gram+diag deferred into the next image's conv1 window so the
            # PE does not stall on the softmax chain
            def _emit_gram(img=img, apad=apad, xft=xft):
                g32 = gp.tile([128, 68], F32, tag="gfin")
                for t in range(32):
                    nc.tensor.matmul(
                        g32, apad[:, t, :, :].rearrange("p a b -> p (a b)"),
                        xft[:, t, :, :].rearrange("p a b -> p (a b)"),
                        start=(t == 0), stop=(t == 31),
                    )
                t0_ = finp.tile([4, 17], F32, tag="t0")
                nc.vector.tensor_copy(t0_, g32[0:4, 0:17])
                t1_ = finp.tile([4, 17], F32, tag="t1")
                nc.vector.tensor_add(t1_, t0_, g32[32:36, 17:34])
                t2_ = finp.tile([4, 17], F32, tag="t2")
                nc.vector.tensor_add(t2_, t1_, g32[64:68, 34:51])
                nc.vector.tensor_add(gall[:, img, :], t2_, g32[96:100, 51:68])
            pending_gram.append(_emit_gram)

        for fn in pending_gram:
            fn()
        pending_gram.clear()
        if debug:
            nc.scalar.dma_start(out=galld, in_=gall)
        # ================= finale: batched over 4 images =================
        cb = finp.tile([4, 4, 16], F32, tag="cb")
        nc.vector.tensor_mul(
            cb,
            cent_sb[:].unsqueeze(1).broadcast_to((4, 4, 16)),
            gall[:, :, 16:17].broadcast_to((4, 4, 16)),
        )
        v4 = finp.tile([4, 4, 16], F32, tag="v4")
        nc.vector.tensor_sub(v4, gall[:, :, 0:16], cb)
        sq = finp.tile([4, 4, 16], F32, tag="sq")
        nc.vector.tensor_mul(sq, v4, v4)
        rs = finp.tile([4, 4, 1], F32, tag="rs")
        nc.vector.reduce_sum(rs, sq, axis=AX.X)
        nrm = finp.tile([4, 4, 1], F32, tag="nrm")
        nc.scalar.activation(out=nrm, in_=rs, func=ACTF.Sqrt)
        rn = finp.tile([4, 4, 1], F32, tag="rn")
        nc.vector.reciprocal(rn, nrm)
        vn = finp.tile([4, 4, 16], F32, tag="vn")
        nc.vector.tensor_mul(vn, v4, rn.broadcast_to((4, 4, 16)))
        sqn = finp.tile([4, 4, 16], F32, tag="sqn")
        nc.vector.tensor_mul(sqn, vn, vn)
        rs2 = finp.tile([4, 4], F32, tag="rs2")
        nc.vector.reduce_sum(rs2, sqn, axis=AX.X)
        # per-image global sums replicated on all 4 partitions
        tps = gp.tile([4, 4], F32, tag="gfin")
        nc.tensor.matmul(tps, ones44_sb[:], rs2[:], start=True, stop=True)
        g1 = finp.tile([4, 4], F32, tag="g1")
        nc.scalar.activation(out=g1, in_=tps, func=ACTF.Sqrt)
        rg = finp.tile([4, 4], F32, tag="rg")
        nc.vector.reciprocal(rg, g1)
        vn2 = finp.tile([4, 4, 16], F32, tag="vn2")
        nc.vector.tensor_mul(
            vn2, vn, rg.unsqueeze(-1).broadcast_to((4, 4, 16)))
        # rearrange to [64=(k,c), img] and apply the linear head
        vcall = finp.tile([64, 4], F32, tag="vcall")
        for i in range(IPC):
            nc.sync.dma_start(
                out=vcall[:, i : i + 1], in_=vn2[:, i, :]
            )
        fps = gp.tile([4, 7], F32, tag="gfin")
        nc.tensor.matmul(fps, vcall[:], wlin_sb[:], start=True, stop=True)
        osb = finp.tile([4, 7], F32, tag="osb")
        nc.vector.tensor_add(osb, fps, linb4_sb[:])
        nc.sync.dma_start(out=out, in_=osb)

    nc.compile()
    return nc


_CACHE = {}


def kernel(x, conv1_w, conv1_b, conv2_w, conv2_b, centroids, assign_w,
           assign_b, lin_w, lin_b):
    # conv biases are zero in this problem; the banded matrices fold weights
    # only, so assert the assumption the kernel relies on.
    assert np.abs(np.asarray(conv1_b)).max() == 0.0
    assert np.abs(np.asarray(conv2_b)).max() == 0.0

    if "nc" not in _CACHE:
        _CACHE["nc"] = _build_program()
    nc = _CACHE["nc"]

    consts = _build_consts(conv1_w, conv2_w, assign_w, assign_b, lin_w, lin_b)
    # xft holds 4x-scaled pooled sums and apad is 0.25-scaled, so the
    # centroid term needs 4*centroids.
    consts["cent"] = 4.0 * np.asarray(centroids, np.float32)
    xr = _round_f32r(np.asarray(x, np.float32))

    in_maps = []
    for c in range(N_CORES):
        mp = dict(consts)
        mp["x"] = np.ascontiguousarray(xr[c * IPC : (c + 1) * IPC])
        in_maps.append(mp)
    res = run_bass_kernel_spmd(nc, in_maps, list(range(N_CORES))).results
    return np.concatenate([res[c]["out"] for c in range(N_CORES)], axis=0)


if __name__ == "__main__":
    print("smoke test: building program only")
    _build_program()
    print("ok")


# revision 29
# speedup vs baseline: 1.0097x; 1.0097x over previous
"""Trainium2 Bass kernel for conv-stack + NetVLAD pooling + linear head.

Pure data parallel: 32 images sharded 4-per-core across 8 NeuronCores.

Structure (per core, 4 images pipelined):
  - x loaded via a few giant dma_starts per image (descriptors spread across
    all 16 DMA engines, ~220 GB/s) into a whole-image SBUF tile
    x1all[(a,ci), 17 tiles, 514].
  - conv1 (3->4, 3x3) as banded f32r matmuls per 32-row tile (3 dx passes,
    512-wide); relu on ACT, w-pool-sum on DVE -> bf16 Y1[128, 17, 258].
  - conv2 (4->16) as banded bf16 matmuls with pool1-h folded into the band;
    odd-phase tiles read from a staged copy x2 built with 4 big strided
    SBUF->SBUF DMAs (split in halves so they overlap conv1).
  - NetVLAD: per 128-w tile ONE bf16 matmul with moving [P | Ablk] computes
    the pair-summed transposed features xft AND the pooled logits
    (pool2 folded into P/Ablk); softmax per 4-tile group; per-tile gram
    accumulated in one PSUM bank via block-padded bf16 lhsT.  The gram of
    image i is emitted inside image i+1's conv1 window so the PE never
    stalls on the softmax chain.
  - Finale (normalize + linear head) batched across the 4 images.
"""
import sys

sys.path.insert(0, "/opt/trn_rl_repo")

import numpy as np
import concourse.bacc as bacc
import concourse.tile as tile
from concourse import mybir
from concourse.bass_utils import run_bass_kernel_spmd

F32 = mybir.dt.float32
F32R = mybir.dt.float32r
BF16 = mybir.dt.bfloat16
AX = mybir.AxisListType
ALU = mybir.AluOpType
ACTF = mybir.ActivationFunctionType

N_CORES = 8
IPC = 4  # images per core
EPS = 1e-12


def _round_f32r(a):
    u = np.ascontiguousarray(a, np.float32).view(np.uint32)
    u = (u + 0x200) & np.uint32(0xFFFFFC00)
    return u.view(np.float32)


def _build_consts(conv1_w, conv2_w, assign_w, assign_b, lin_w, lin_b):
    c1w = np.asarray(conv1_w, np.float32)
    c2w = np.asarray(conv2_w, np.float32)
    # conv1 banded lhsT, 3 variants (var*3+dx slots):
    #  var0 (mid, tiles r=1..15): x anchor h=32r-3+a, outputs j=h_out-(32r-2)
    #    rows p=(j+dy)*3+ci, cols q=j*4+co
    #  var1 (r=0): x anchor h=a, outputs cols q=j*4+co with h_out=j-2
    #    (cols j<2 zero), taps at a=j-3+dy >= 0
    #  var2 (r=16): x anchor h=478+a, outputs h_out=510+jj at cols jj*4+co,
    #    taps at a=31+jj+dy <= 33
    W1 = np.zeros((102, 9, 128), np.float32)
    for dx in range(3):
        for co in range(4):
            for ci in range(3):
                for dy in range(3):
                    for j in range(32):
                        W1[(j + dy) * 3 + ci, dx, j * 4 + co] = c1w[co, ci, dy, dx]
                    for j in range(2, 32):
                        a = j - 3 + dy
                        if a >= 0:
                            W1[a * 3 + ci, 3 + dx, j * 4 + co] = c1w[co, ci, dy, dx]
                    for jj in range(2):
                        a = 31 + jj + dy
                        if a <= 33:
                            W1[a * 3 + ci, 6 + dx, jj * 4 + co] = c1w[co, ci, dy, dx]
    # conv2 banded lhsT with pool1-h fold (input rows are unpooled Y1 rows)
    # and the w-pools stored as SUMs: total scale 0.25.
    W2 = np.zeros((80, 3, 128), np.float32)
    for dx in range(3):
        for co in range(16):
            for ci in range(4):
                for dy in range(3):
                    for rr in range(8):
                        for half in range(2):
                            W2[(2 * rr + 2 * dy + half) * 4 + ci, dx, rr * 16 + co] = (
                                0.25 * c2w[co, ci, dy, dx]
                            )
    # combined moving matrix M = [P | Ablk]:
    #  cols 0:64  : P[(2q+h)*16+c, q*16+c] = 1   (xft pair-sum)
    #  cols 64:80 : Ablk[(2q+h)*16+c, q*4+k] = 0.25*aw[k,c]  (pooled logits)
    aw = np.asarray(assign_w, np.float32)
    M = np.zeros((128, 80), np.float32)
    for q in range(4):
        for c in range(16):
            for h in range(2):
                M[(2 * q + h) * 16 + c, q * 16 + c] = 1.0
                for k in range(4):
                    M[(2 * q + h) * 16 + c, 64 + q * 4 + k] = 0.25 * aw[k, c]
    brep = np.tile(np.asarray(assign_b, np.float32), 4).reshape(16)
    brep128 = np.broadcast_to(brep, (128, 16)).copy()
    linb4 = np.broadcast_to(np.asarray(lin_b, np.float32), (4, 7)).copy()
    return {
        "w1": _round_f32r(W1),
        "w2": _round_f32r(W2),
        "m": _round_f32r(M),
        "brep": brep128.astype(np.float32),
        "cent": np.zeros(0),  # set by caller (4x centroids)
        "wlin": np.asarray(lin_w, np.float32).T.copy(),  # (64, 7)
        "linb4": linb4,
        "ones41": np.ones((4, 1), np.float32),
        "ones44": np.ones((4, 4), np.float32),
    }


def _build_program(debug=False):
    nc = bacc.Bacc("TRN2", target_bir_lowering=False, debug=False,
                   num_devices=N_CORES)
    xin = nc.dram_tensor("x", [IPC, 3, 512, 512], F32R, kind="ExternalInput").ap()
    w1 = nc.dram_tensor("w1", [102, 9, 128], F32R, kind="ExternalInput").ap()
    w2 = nc.dram_tensor("w2", [80, 3, 128], F32R, kind="ExternalInput").ap()
    m_in = nc.dram_tensor("m", [128, 80], F32R, kind="ExternalInput").ap()
    brep = nc.dram_tensor("brep", [128, 16], F32, kind="ExternalInput").ap()
    cent = nc.dram_tensor("cent", [4, 16], F32, kind="ExternalInput").ap()
    wlin = nc.dram_tensor("wlin", [64, 7], F32, kind="ExternalInput").ap()
    linb4 = nc.dram_tensor("linb4", [4, 7], F32, kind="ExternalInput").ap()
    ones41 = nc.dram_tensor("ones41", [4, 1], F32, kind="ExternalInput").ap()
    ones44 = nc.dram_tensor("ones44", [4, 4], F32, kind="ExternalInput").ap()
    out = nc.dram_tensor("out", [IPC, 7], F32, kind="ExternalOutput").ap()
    if debug:
        y1d = nc.dram_tensor("y1d", [IPC, 128, 17, 258], F32R, kind="ExternalOutput").ap()
        vd = nc.dram_tensor("vd", [IPC, 128, 32, 128], F32R, kind="ExternalOutput").ap()
        xad = nc.dram_tensor("xad", [IPC, 102, 17, 514], F32R, kind="ExternalOutput").ap()
        lbd = nc.dram_tensor("lbd", [IPC, 128, 32, 16], F32, kind="ExternalOutput").ap()
        galld = nc.dram_tensor("galld", [4, 4, 17], F32, kind="ExternalOutput").ap()

    from contextlib import ExitStack

    with tile.TileContext(nc) as tc, ExitStack() as es:
        consts = es.enter_context(tc.tile_pool(name="consts", bufs=1))
        vp = es.enter_context(tc.tile_pool(name="vp", bufs=2))
        re1p = es.enter_context(tc.tile_pool(name="re1p", bufs=2))
        re2p = es.enter_context(tc.tile_pool(name="re2p", bufs=4))
        smp = es.enter_context(tc.tile_pool(name="smp", bufs=2))
        finp = es.enter_context(tc.tile_pool(name="finp", bufs=1))
        cvp = es.enter_context(tc.tile_pool(name="cvp", bufs=2, space="PSUM"))
        xltp = es.enter_context(tc.tile_pool(name="xltp", bufs=2, space="PSUM"))
        gp = es.enter_context(tc.tile_pool(name="gp", bufs=1, space="PSUM"))

        w1_sb = consts.tile([102, 9, 128], F32R)
        nc.scalar.dma_start(out=w1_sb, in_=w1)
        w2_sb = consts.tile([80, 3, 128], F32R)
        nc.scalar.dma_start(out=w2_sb, in_=w2)
        w2bf_sb = consts.tile([80, 3, 128], BF16)
        nc.vector.tensor_copy(w2bf_sb, w2_sb.bitcast(F32))
        m_sb = consts.tile([128, 80], F32R)
        nc.scalar.dma_start(out=m_sb, in_=m_in)
        mbf_sb = consts.tile([128, 80], BF16)
        nc.vector.tensor_copy(mbf_sb, m_sb.bitcast(F32))
        brep_sb = consts.tile([128, 16], F32)
        nc.scalar.dma_start(out=brep_sb, in_=brep)
        cent_sb = consts.tile([4, 16], F32)
        nc.scalar.dma_start(out=cent_sb, in_=cent)
        wlin_sb = consts.tile([64, 7], F32)
        nc.scalar.dma_start(out=wlin_sb, in_=wlin)
        linb4_sb = consts.tile([4, 7], F32)
        nc.scalar.dma_start(out=linb4_sb, in_=linb4)
        ones41_sb = consts.tile([4, 1], F32)
        nc.scalar.dma_start(out=ones41_sb, in_=ones41)
        ones44_sb = consts.tile([4, 4], F32)
        nc.scalar.dma_start(out=ones44_sb, in_=ones44)

        # persistent tiles with pre-initialized constant regions
        xa = consts.tile([102, 17, 514], F32R, tag="xa")
        nc.vector.memset(xa[:, :, 0:1].bitcast(F32), 0.0)
        nc.vector.memset(xa[:, :, 513:514].bitcast(F32), 0.0)
        xav = xa.rearrange("(a c) r w -> a c r w", c=3)
        xfts = []
        apads = []
        for half in range(2):
            xft = consts.tile([128, 32, 4, 17], BF16, tag=f"xft{half}")
            nc.vector.memset(xft[:, :, :, 16:17], 1.0)
            xfts.append(xft)
            apad = consts.tile([128, 32, 4, 32], BF16, tag=f"apad{half}")
            nc.vector.memset(apad[:, :, :, 4:32], 0.0)
            apads.append(apad)

        x2 = consts.tile([80, 16, 258], BF16, tag="x2")
        y1s = []
        for half in range(2):
            y1t = consts.tile([128, 17, 258], BF16, tag=f"y1{half}")
            nc.vector.memset(y1t[:, :, 0:1], 0.0)
            nc.vector.memset(y1t[:, :, 257:258], 0.0)
            y1s.append(y1t)

        gall = finp.tile([4, 4, 17], F32, tag="gall")
        pending_gram = []

        for img in range(IPC):
            # ================== x load: 4 giant DMAs ==================
            # r=0 tile: anchor h=a (rows 0..33)
            for ci in range(3):
                nc.sync.dma_start(
                    out=xav[:, ci, 0, 1:513], in_=xin[img, ci, 0:34, :]
                )
            # mid tiles r=1..15, a=2..33: rows 31..510 partitioned exactly
            # (DMA APs are limited to 3 dims, so one dma_start per channel)
            chunks = [(1, 4), (4, 16)] if img == 0 else [(1, 16)]
            for ci in range(3):
                for lo, hi in chunks:
                    nc.sync.dma_start(
                        out=xav[2:34, ci, lo:hi, 1:513],
                        in_=xin[img, ci, 32 * lo - 1 : 32 * (hi - 1) + 31, :]
                        .rearrange("(r a) w -> a r w", a=32),
                    )
                # mid tiles r=1..15, a=0..1: rows 32r-3, 32r-2
                nc.sync.dma_start(
                    out=xav[0:2, ci, 1:16, 1:513],
                    in_=xin[img, ci, 29:509, :].rearrange("(r k) w -> k r w", k=32)[0:2],
                )
            # r=16 tile: anchor h=478+a
            for ci in range(3):
                nc.sync.dma_start(
                    out=xav[:, ci, 16, 1:513], in_=xin[img, ci, 478:512, :]
                )

            # ================== conv1: 8 pairs + tail ==================
            y1 = y1s[img % 2]
            for pi in range(8):
                p1 = cvp.tile([128, 2, 512], F32, tag="cv")
                for j in range(2):
                    r = 2 * pi + j
                    var = 1 if r == 0 else 0
                    for dx in range(3):
                        nc.tensor.matmul(
                            p1[:, j, :], w1_sb[:, var * 3 + dx, :],
                            xa[:, r, dx : dx + 512],
                            start=(dx == 0), stop=(dx == 2),
                        )
                re1 = re1p.tile([128, 2, 512], F32, tag="re1")
                nc.scalar.activation(out=re1, in_=p1, func=ACTF.Relu)
                re1v = re1.rearrange("p a (w two) -> p a w two", two=2)
                nc.vector.tensor_add(
                    y1[:, 2 * pi : 2 * pi + 2, 1:257],
                    re1v[:, :, :, 0], re1v[:, :, :, 1],
                )
                if pi == 4:
                    # first staging half: x2 slots 0..7 need y1 blocks 0..8
                    nc.sync.dma_start(out=x2[0:64, 0:8, :], in_=y1[64:128, 0:8, :])
                    nc.sync.dma_start(out=x2[64:80, 0:8, :], in_=y1[0:16, 1:9, :])
            # tail tile r=16 (writes full 128 partitions: zeros outside rows
            # 510/511, giving conv2 its zero padding rows)
            p1t = cvp.tile([128, 2, 512], F32, tag="cv")
            for dx in range(3):
                nc.tensor.matmul(
                    p1t[:, 0, :], w1_sb[:, 6 + dx, :], xa[:, 16, dx : dx + 512],
                    start=(dx == 0), stop=(dx == 2),
                )
            re1t = re1p.tile([128, 2, 512], F32, tag="re1")
            nc.scalar.activation(out=re1t[:, 0, :], in_=p1t[:, 0, :], func=ACTF.Relu)
            re1tv = re1t.rearrange("p a (w two) -> p a w two", two=2)
            nc.vector.tensor_add(
                y1[:, 16:17, 1:257], re1tv[:, 0:1, :, 0], re1tv[:, 0:1, :, 1]
            )
            # second staging half: x2 slots 8..15 need y1 blocks 8..16
            nc.sync.dma_start(out=x2[0:64, 8:16, :], in_=y1[64:128, 8:16, :])
            nc.sync.dma_start(out=x2[64:80, 8:16, :], in_=y1[0:16, 9:17, :])

            # ========= conv2 (8 groups) interleaved with NetVLAD xlt =========
            v = vp.tile([128, 32, 128], BF16, tag="v")
            v2v = v.rearrange("p (t two) w -> p t two w", two=2)
            xft = xfts[img % 2]
            apad = apads[img % 2]
            lb = smp.tile([128, 32, 16], F32, tag="lb")

            def conv2_group(g, parity):
                p2 = cvp.tile([128, 2, 512], F32, tag="cv")
                p2v = p2.rearrange("p a (b w two) -> p (a b) w two", two=2, b=2)
                rhs = (y1[0:80] if parity == 0 else x2)
                for pr in range(2):
                    blk = 4 * g + 2 * pr
                    for dx in range(3):
                        nc.tensor.matmul(
                            p2.rearrange("p a (b w) -> p (a b) w", b=2)[
                                :, 2 * pr : 2 * pr + 2, :],
                            w2bf_sb[:, dx, :],
                            rhs[:, blk : blk + 2, dx : dx + 256],
                            start=(dx == 0), stop=(dx == 2),
                        )
                re2 = re2p.tile([128, 4, 128], F32, tag="re2")
                nc.scalar.activation(out=re2, in_=p2v[:, :, :, 0], func=ACTF.Relu)
                nc.vector.scalar_tensor_tensor(
                    out=v2v[:, 4 * g : 4 * g + 4, parity, :],
                    in0=p2v[:, :, :, 1], scalar=0.0, in1=re2,
                    op0=ALU.max, op1=ALU.add,
                )

            def xlt_group(g8):
                xlt = xltp.tile([128, 4, 80], F32, tag="xlt")
                for tt in range(4):
                    nc.tensor.matmul(
                        xlt[:, tt, :], v[:, 4 * g8 + tt, :], mbf_sb[:],
                        start=True, stop=True,
                    )
                nc.vector.tensor_copy(
                    xft[:, 4 * g8 : 4 * g8 + 4, :, 0:16],
                    xlt[:, :, 0:64].rearrange("p t (q c) -> p t q c", q=4),
                )
                nc.vector.tensor_add(
                    lb[:, 4 * g8 : 4 * g8 + 4, :],
                    xlt[:, :, 64:80],
                    brep_sb[:].unsqueeze(1).broadcast_to((128, 4, 16)),
                )

            def softmax_group(g8):
                # tiles 4*g8..4*g8+3, right after their lb is produced
                lbv = lb[:, 4 * g8 : 4 * g8 + 4, :].rearrange(
                    "p t (q k) -> p t q k", k=4)
                mx = smp.tile([128, 4, 4], F32, tag="mx")
                nc.vector.reduce_max(mx, lbv, axis=AX.X)
                ls = smp.tile([128, 4, 4, 4], F32, tag="ls")
                nc.vector.tensor_sub(
                    ls, lbv, mx.unsqueeze(-1).broadcast_to((128, 4, 4, 4)))
                ae = smp.tile([128, 4, 4, 4], F32, tag="ae")
                nc.scalar.activation(out=ae, in_=ls, func=ACTF.Exp)
                zs = smp.tile([128, 4, 4], F32, tag="zs")
                nc.vector.reduce_sum(zs, ae, axis=AX.X)
                rz = smp.tile([128, 4, 4], F32, tag="rz")
                nc.vector.reciprocal(rz, zs)
                nc.vector.scalar_tensor_tensor(
                    out=apad[:, 4 * g8 : 4 * g8 + 4, :, 0:4], in0=ae,
                    scalar=0.25,
                    in1=rz.unsqueeze(-1).broadcast_to((128, 4, 4, 4)),
                    op0=ALU.mult, op1=ALU.mult,
                )

            conv2_group(0, 0)
            conv2_group(0, 1)
            # flush previous image's deferred gram here: apad of image i-1 is
            # ready by now and conv1+2 groups keep the PE fed meanwhile
            for fn in pending_gram:
                fn()
            pending_gram.clear()
            xlt_group(0)
            softmax_group(0)
            xlt_group(1)
            softmax_group(1)
            conv2_group(1, 0)
            conv2_group(1, 1)
            xlt_group(2)
            softmax_group(2)
            xlt_group(3)
            softmax_group(3)
            conv2_group(2, 0)
            conv2_group(2, 1)
            xlt_group(4)
            softmax_group(4)
            xlt_group(5)
            softmax_group(5)
            conv2_group(3, 0)
            conv2_group(3, 1)
            xlt_group(6)
            softmax_group(6)
            xlt_group(7)
            softmax_group(7)
            if debug:
                nc.scalar.dma_start(out=xad[img], in_=xa)
                nc.scalar.dma_start(out=y1d[img], in_=y1)
                nc.scalar.dma_start(out=vd[img], in_=v)
                nc.scalar.dma_start(out=lbd[img], in_=lb)

            # gram+diag deferred into the next image's conv1 window so the
            # PE does not stall on the softmax chain
            def _emit_gram(img=img, apad=apad, xft=xft):
                g32 = gp.tile([128, 68], F32, tag="gfin")
                for t in range(32):
                    nc.tensor.matmul(
                        g32, apad[:, t, :, :].rearrange("p a b -> p (a b)"),
                        xft[:, t, :, :].rearrange("p a b -> p (a b)"),
                        start=(t == 0), stop=(t == 31),
                    )
                t0_ = finp.tile([4, 17], F32, tag="t0")
                nc.vector.tensor_copy(t0_, g32[0:4, 0:17])
                t1_ = finp.tile([4, 17], F32, tag="t1")
                nc.vector.tensor_add(t1_, t0_, g32[32:36, 17:34])
                t2_ = finp.tile([4, 17], F32, tag="t2")
                nc.vector.tensor_add(t2_, t1_, g32[64:68, 34:51])
                nc.vector.tensor_add(gall[:, img, :], t2_, g32[96:100, 51:68])
            pending_gram.append(_emit_gram)

        for fn in pending_gram:
            fn()
        pending_gram.clear()
        if debug:
            nc.scalar.dma_start(out=galld, in_=gall)
        # ================= finale: batched over 4 images =================
        cb = finp.tile([4, 4, 16], F32, tag="cb")
        nc.vector.tensor_mul(
            cb,
            cent_sb[:].unsqueeze(1).broadcast_to((4, 4, 16)),
            gall[:, :, 16:17].broadcast_to((4, 4, 16)),
        )
        v4 = finp.tile([4, 4, 16], F32, tag="v4")
        nc.vector.tensor_sub(v4, gall[:, :, 0:16], cb)
        sq = finp.tile([4, 4, 16], F32, tag="sq")
        nc.vector.tensor_mul(sq, v4, v4)
        rs = finp.tile([4, 4, 1], F32, tag="rs")
        nc.vector.reduce_sum(rs, sq, axis=AX.X)
        nrm = finp.tile([4, 4, 1], F32, tag="nrm")
        nc.scalar.activation(out=nrm, in_=rs, func=ACTF.Sqrt)
        rn = finp.tile([4, 4, 1], F32, tag="rn")
        nc.vector.reciprocal(rn, nrm)
        vn = finp.tile([4, 4, 16], F32, tag="vn")
        nc.vector.tensor_mul(vn, v4, rn.broadcast_to((4, 4, 16)))
        sqn = finp.tile([4, 4, 16], F32, tag="sqn")
        nc.vector.tensor_mul(sqn, vn, vn)
        rs2 = finp.tile([4, 4], F32, tag="rs2")
        nc.vector.reduce_sum(rs2, sqn, axis=AX.X)
        # per-image global sums replicated on all 4 partitions
        tps = gp.tile([4, 4], F32, tag="gfin")
        nc.tensor.matmul(tps, ones44_sb[:], rs2[:], start=True, stop=True)
        g1 = finp.tile([4, 4], F32, tag="g1")
        nc.scalar.activation(out=g1, in_=tps, func=ACTF.Sqrt)
        rg = finp.tile([4, 4], F32, tag="rg")
        nc.vector.reciprocal(rg, g1)
        vn2 = finp.tile([4, 4, 16], F32, tag="vn2")
        nc.vector.tensor_mul(
            vn2, vn, rg.unsqueeze(-1).broadcast_to((4, 4, 16)))
        # rearrange to [64=(k,c), img] and apply the linear head
        vcall = finp.tile([64, 4], F32, tag="vcall")
        for i in range(IPC):
            nc.sync.dma_start(
                out=vcall[:, i : i + 1], in_=vn2[:, i, :]
            )
        fps = gp.tile([4, 7], F32, tag="gfin")
        nc.tensor.matmul(fps, vcall[:], wlin_sb[:], start=True, stop=True)
        osb = finp.tile([4, 7], F32, tag="osb")
        nc.vector.tensor_add(osb, fps, linb4_sb[:])
        nc.sync.dma_start(out=out, in_=osb)

    nc.compile()
    return nc


_CACHE = {}


def kernel(x, conv1_w, conv1_b, conv2_w, conv2_b, centroids, assign_w,
           assign_b, lin_w, lin_b):
    # conv biases are zero in this problem; the banded matrices fold weights
    # only, so assert the assumption the kernel relies on.
    assert np.abs(np.asarray(conv1_b)).max() == 0.0
    assert np.abs(np.asarray(conv2_b)).max() == 0.0

    if "nc" not in _CACHE:
        _CACHE["nc"] = _build_program()
    nc = _CACHE["nc"]

    consts = _build_consts(conv1_w, conv2_w, assign_w, assign_b, lin_w, lin_b)
    # xft holds 4x-scaled pooled sums and apad is 0.25-scaled, so the
    # centroid term needs 4*centroids.
    consts["cent"] = 4.0 * np.asarray(centroids, np.float32)
    xr = _round_f32r(np.asarray(x, np.float32))

    in_maps = []
    for c in range(N_CORES):
        mp = dict(consts)
        mp["x"] = np.ascontiguousarray(xr[c * IPC : (c + 1) * IPC])
        in_maps.append(mp)
    res = run_bass_kernel_spmd(nc, in_maps, list(range(N_CORES))).results
    return np.concatenate([res[c]["out"] for c in range(N_CORES)], axis=0)


if __name__ == "__main__":
    print("smoke test: building program only")
    _build_program()
    print("ok")
